# revision 59
# baseline (speedup 1.0000x reference)
"""Trainium2 Bass kernel for a 3-layer minLSTM-style NLP model.

Model (per reference):
  x = emb[ids]                                   (B,S,E) = (2,2048,512)
  3 x { xn = LN(x); gates = xn @ Ws.T + bs;
        f' = sig(f)/(sig(f)+sig(i)); i' = 1-f';
        v = i' * g(tilde), g(x) = max(x+0.5, sigmoid(x));
        h_t = f'_t h_{t-1} + v_t  (h_0 = 0.5);  x = h + x }
  xf = LN(x) * fln_w;  logits = xf @ fc_w.T + fc_b    (B,S,32000)

Sharding (8 cores, zero collectives):
  core c -> (batch b=c//4, seq chunk j=c%4 of 512 tokens). Each core runs a
  640-token window (128-token halo before its own 512) through the recurrent
  stack; the forget-product decays the unknown initial state to ~0 over the
  halo, and a per-core reset constant makes j==0 exact at the batch start.
  Each core computes logits for its own 512 tokens against the full vocab.

Key layout tricks:
  - LN affine (ln_w/ln_b) folded into the gate weights/biases on host, and
    fln_w folded into fc_w, so the device only applies (x-m)*rstd.
  - Per-token LN stats computed TRANSPOSED (tokens on partitions) via 1-row
    matmuls; rsqrt via magic-constant Newton on tiny [128,5] tiles; the
    (rstd, -m*rstd) stats transposed back in ONE PE transpose and broadcast
    across partitions by the Pool engine.
  - Mean correction applied as xn = x*rstd + bcast(-m*rstd) on DVE (no
    rank-1 GEMM update).
  - Gate math: f' = sf/(sf+si) via a single DVE divide; v-side fused with
    scalar_tensor_tensor so the scan consumes (f'-1)*g with op1=subtract.
  - fc_w streamed as fp8 e4m3 hi+residual (3-pass residual-corrected GEMM in
    DoubleRow mode); the 32.8MB weight stream is prefetched on a dedicated
    DMA queue starting at t=0 so phase C stays PE-bound.
  - Activations f16 everywhere; output logits written f16 and upcast on host.
"""

import sys

if "/opt/trn_rl_repo" not in sys.path:
    sys.path.insert(0, "/opt/trn_rl_repo")

import numpy as np

import concourse.bass as bass
import concourse.bacc as bacc
import concourse.tile as tile
from concourse import mybir
from concourse.bass import IndirectOffsetOnAxis
from concourse.bass_utils import run_bass_kernel_spmd
from concourse.masks import make_identity

F32 = mybir.dt.float32
F16 = mybir.dt.float16
F8 = mybir.dt.float8e4
I32 = mybir.dt.int32
AF = mybir.ActivationFunctionType
OP = mybir.AluOpType

# problem constants
B, S, V, H, L = 2, 2048, 32000, 512, 3
P = 128
KT = H // P            # 4 k-tiles over the H contraction dim
CHUNK = 512            # own tokens per core
HALO = 128             # speculative scan warmup tokens
W = HALO + CHUNK       # 640 window tokens per core
NG = W // P            # 5 embedding gather groups
NCH = [(0, 512), (512, 128)]   # window free-dim chunks (PSUM-bounded)
N_CORES = 8
EPS = 1e-5
MAGIC2 = 0x1EF759DF    # rsqrt seed magic, pre-adjusted for hneg=-(var+eps)/2


def build_program(fcwb=11, psgb=3, pstb=3, workb=2, fa=8, fb=4, fc=12, u_eng="act", prefetch=True):
    nc = bacc.Bacc("TRN2", target_bir_lowering=False, debug=False,
                   enable_asserts=True, num_devices=N_CORES)

    idx_t = nc.dram_tensor("idx", [P, NG], I32, kind="ExternalInput").ap()
    emb_t = nc.dram_tensor("emb", [V, H], F16, kind="ExternalInput").ap()
    wsT_t = nc.dram_tensor("wsT", [L, KT, P, 3 * H], F16, kind="ExternalInput").ap()
    bsg_t = nc.dram_tensor("bsg", [P, L * 16], F32, kind="ExternalInput").ap()
    fcwt_t = nc.dram_tensor("fcwt", [25, P, 10, 2, 2, 2, P], F8, kind="ExternalInput").ap()
    fcb_t = nc.dram_tensor("fcb", [P, V // P], F32, kind="ExternalInput").ap()
    rst_t = nc.dram_tensor("rst", [P, 2], F32, kind="ExternalInput").ap()
    out_t = nc.dram_tensor("out", [P, V // (2 * P), 2, CHUNK], F16,
                           kind="ExternalOutput").ap()

    with tile.TileContext(nc) as tc:
        with tc.tile_pool(name="singles", bufs=1) as singles, \
             tc.tile_pool(name="persist", bufs=1) as persist, \
             tc.tile_pool(name="wst", bufs=2) as wstp, \
             tc.tile_pool(name="fcw", bufs=fcwb) as fcwp:

            # ---- constants / small inputs (scalar queue: fast dispatch) ----
            idx = singles.tile([P, NG], I32)
            nc.scalar.dma_start(out=idx[:], in_=idx_t[:])
            bsg = singles.tile([P, L * 16], F32)
            nc.scalar.dma_start(out=bsg[:], in_=bsg_t[:])
            rst = singles.tile([P, 2], F32)
            nc.scalar.dma_start(out=rst[:], in_=rst_t[:])
            fcb2 = singles.tile([P, V // P], F32)
            nc.scalar.dma_start(out=fcb2[:], in_=fcb_t[:])
            ident16 = singles.tile([P, P], F16)
            make_identity(nc, ident16[:])
            actwarm = singles.tile([1, 1], F32)
            nc.scalar.activation(out=actwarm[:], in_=rst[0:1, 0:1],
                                 func=AF.Sigmoid)
            ones16 = singles.tile([P, 1], F16)   # stats-reduce rhs
            nc.vector.memset(ones16[:], 1.0)

            # ---- gate weights: 2-deep rotation; l0/l1 up front (sync queue,
            # ahead of the fcw stream), l2 JIT on the Pool queue during l0 ----
            def load_wst(l, queue):
                w = wstp.tile([P, KT * 3 * H], F16, tag="wst", name=f"wst{l}")
                for kk in range(KT):
                    queue.dma_start(
                        out=w[:, kk * 3 * H:(kk + 1) * 3 * H],
                        in_=wsT_t[l, kk])
                return w

            wst = [load_wst(0, nc.sync), load_wst(1, nc.sync), None]

            # ---- fc_w fp8 stream: issue every load now; the pool's buf
            # rotation throttles against phase C's consumption ----
            fcw_tiles = []
            if prefetch:
                for vg in range(25):
                    fcw = fcwp.tile([P, 10, 2, 2, 2, P], F8, tag="fcw")
                    nc.sync.dma_start(out=fcw[:], in_=fcwt_t[vg])
                    fcw_tiles.append(fcw)

            # final activations (channel-major), consumed by phase C
            xf_bf = [persist.tile([P, CHUNK], F16, tag=f"xfbf{k}", name=f"xfbf{k}")
                     for k in range(KT)]

            with tc.tile_pool(name="xpool", bufs=2) as xpool, \
                 tc.tile_pool(name="work", bufs=workb) as work, \
                 tc.tile_pool(name="scan", bufs=1) as scanp, \
                 tc.tile_pool(name="xnp", bufs=1) as xnp, \
                 tc.tile_pool(name="bc", bufs=1) as bcp, \
                 tc.tile_pool(name="stat", bufs=1) as statp, \
                 tc.tile_pool(name="psg", bufs=psgb, space="PSUM") as psg, \
                 tc.tile_pool(name="pss", bufs=1, space="PSUM") as pss, \
                 tc.tile_pool(name="psgr", bufs=1, space="PSUM") as psgr, \
                 tc.tile_pool(name="pst", bufs=pstb, space="PSUM") as pst:

                # ---- phase A: embedding gather + transpose to channel-major
                x = [xpool.tile([P, W], F16, tag=f"x{k}", name=f"xt{k}")
                     for k in range(KT)]
                with tc.tile_pool(name="gath", bufs=1) as gathp:
                    xgs = []
                    for g in range(NG):
                        xg = gathp.tile([P, H], F16, tag=f"xg{g}", name=f"xg{g}")
                        nc.gpsimd.indirect_dma_start(
                            out=xg[:], out_offset=None, in_=emb_t[:],
                            in_offset=IndirectOffsetOnAxis(ap=idx[:, g:g + 1], axis=0),
                        )
                        xgs.append(xg)
                    for g in range(NG):
                        xg = xgs[g]
                        for k in range(KT):
                            ptr = pst.tile([P, P], F16, tag="pstt", name="ptr")
                            nc.tensor.transpose(
                                out=ptr[:], in_=xg[:, k * P:(k + 1) * P],
                                identity=ident16[:])
                            eng = nc.vector if (g * KT + k) % 2 == 0 else nc.scalar
                            if eng is nc.vector:
                                nc.vector.tensor_copy(
                                    out=x[k][:, g * P:(g + 1) * P], in_=ptr[:])
                            else:
                                nc.scalar.copy(
                                    out=x[k][:, g * P:(g + 1) * P], in_=ptr[:])

                # ---- helper: transposed LN stats + rsqrt newton ----
                def ln_stats(xs, g0, ngr, tag):
                    """Per-token rstd and -mean*rstd for token groups
                    [g0, g0+ngr): returned as a [2*ngr, P] f16 tile whose row
                    q holds rstd (q < ngr) / -m*rstd (q >= ngr) for group q."""
                    psT = pss.tile([P, 8], F32, tag="psT", name="psT")
                    sums_b = statp.tile([P, ngr], F32, tag="sumb")
                    scr = statp.tile([P, P], F32, tag="ttrscr")
                    # 4 gram slots in one PSUM bank so group g+1's matmuls
                    # overlap group g's diag extraction
                    gram = psgr.tile([P, 4 * P], F32, tag="gram", name="gram")
                    for g in range(ngr):
                        sl = slice((g0 + g) * P, (g0 + g + 1) * P)
                        gsl = slice((g % 4) * P, (g % 4 + 1) * P)
                        for k in range(KT):
                            nc.tensor.matmul(
                                out=psT[:, g:g + 1], lhsT=xs[k][:, sl],
                                rhs=ones16[:],
                                start=(k == 0), stop=(k == KT - 1))
                        for k in range(KT):
                            nc.tensor.matmul(
                                out=gram[:, gsl], lhsT=xs[k][:, sl],
                                rhs=xs[k][:, sl],
                                start=(k == 0), stop=(k == KT - 1))
                        # sum_x2 = diag(gram) via (gram * I) row-reduce
                        nc.vector.tensor_tensor(
                            out=scr[:], in0=gram[:, gsl], in1=ident16[:],
                            op=OP.mult)
                        nc.vector.tensor_reduce(
                            out=sums_b[:, g:g + 1], in_=scr[:], op=OP.add,
                            axis=mybir.AxisListType.X)
                    m2 = statp.tile([P, ngr], F32, tag="m2")
                    hneg = statp.tile([P, ngr], F32, tag="hneg")
                    y = statp.tile([P, ngr], F32, tag="y")
                    t = statp.tile([P, ngr], F32, tag="t")
                    rsmr = statp.tile([P, 2 * ngr], F16, tag="rsmr")
                    # m2 = sum_x^2/(2 H^2) = m^2/2, all-DVE: stage the PSUM
                    # sums into SBUF first (s2s2d2 forbids two PSUM reads)
                    psTs = statp.tile([P, ngr], F32, tag="psTs")
                    nc.vector.tensor_copy(out=psTs[:], in_=psT[:, 0:ngr])
                    nc.vector.scalar_tensor_tensor(
                        out=m2[:], in0=psTs[:], scalar=0.5 / (H * H),
                        in1=psTs[:], op0=OP.mult, op1=OP.mult)
                    # hneg = m^2/2 - (sum_x2/(2H) + eps/2) = -(var+eps)/2
                    nc.vector.tensor_scalar(
                        out=hneg[:], in0=sums_b[:],
                        scalar1=0.5 / H, scalar2=EPS / 2,
                        op0=OP.mult, op1=OP.add)
                    nc.vector.tensor_sub(hneg[:], m2[:], hneg[:])
                    # rsqrt seed: y = -( (bits(hneg)>>1) - MAGIC2 )
                    nc.vector.tensor_scalar(
                        out=y[:].bitcast(I32), in0=hneg[:].bitcast(I32),
                        scalar1=1, scalar2=None,
                        op0=OP.arith_shift_right)
                    nc.vector.tensor_scalar(
                        out=y[:].bitcast(I32), in0=y[:].bitcast(I32),
                        scalar1=MAGIC2, scalar2=-1, op0=OP.subtract,
                        op1=OP.mult)
                    nc.vector.tensor_mul(t[:], y[:], y[:])
                    nc.vector.tensor_mul(t[:], t[:], hneg[:])
                    nc.vector.scalar_tensor_tensor(
                        out=y[:], in0=t[:], scalar=1.5, in1=y[:],
                        op0=OP.add, op1=OP.mult)
                    nc.vector.tensor_copy(out=rsmr[:, 0:ngr], in_=y[:])
                    # mr = -(sum_x/H)*rstd
                    nc.vector.scalar_tensor_tensor(
                        out=rsmr[:, ngr:2 * ngr], in0=psT[:, 0:ngr],
                        scalar=-1.0 / H, in1=y[:], op0=OP.mult, op1=OP.mult)
                    # transpose each column separately so every row lands
                    # on partition 0 (partition_broadcast requirement)
                    rows = statp.tile([1, 2 * NG * P], F16, tag=f"rows{tag}",
                                      name=f"rows{tag}")
                    for q in range(2 * ngr):
                        ptrq = pst.tile([1, P], F16, tag="pstt", name="ptrq")
                        nc.tensor.transpose(out=ptrq[:], in_=rsmr[:, q:q + 1],
                                            identity=ident16[:])
                        nc.scalar.copy(
                            out=rows[0:1, q * P:(q + 1) * P], in_=ptrq[:])
                    return rows

                def pe_filler(n, rhs_tile):
                    """Keep the PE p-state ramp warm with dead matmuls into
                    a rotating PSUM tile nobody reads."""
                    for _ in range(n):
                        pg = psg.tile([P, 512], F32, tag="pg", name="fill")
                        nc.tensor.matmul(out=pg[:], lhsT=ident16[:],
                                         rhs=rhs_tile[:, 0:512],
                                         start=True, stop=True)

                # ---- phase B: L recurrent layers ----
                for l in range(L):
                    if l == 0:
                        wst[2] = load_wst(2, nc.gpsimd)
                    rows = ln_stats(x, 0, NG, "b")
                    if fa:
                        pe_filler(fa, x[0])

                    # broadcast rstd / -m*rstd across partitions (Pool);
                    # mrb broadcasts overlap the xn multiplies
                    rb = bcp.tile([P, W], F16, tag="rb")
                    mrb = bcp.tile([P, W], F16, tag="mrb")
                    for g in range(NG):
                        nc.gpsimd.partition_broadcast(
                            rb[:, g * P:(g + 1) * P],
                            rows[0:1, g * P:(g + 1) * P])
                    # xn = x*rstd - m*rstd
                    xn = [xnp.tile([P, W], F16, tag=f"xn{k}", name=f"xn{k}")
                          for k in range(KT)]
                    for k in range(KT):
                        nc.vector.tensor_mul(xn[k][:], x[k][:], rb[:])
                    for g in range(NG):
                        nc.gpsimd.partition_broadcast(
                            mrb[:, g * P:(g + 1) * P],
                            rows[0:1, (NG + g) * P:(NG + g + 1) * P])
                    for k in range(KT):
                        nc.vector.tensor_add(xn[k][:], xn[k][:], mrb[:])
                    if fb:
                        pe_filler(fb, x[0])

                    # --- gates GEMM + nonlinearities + scan ---
                    x2 = [xpool.tile([P, W], F16, tag=f"x{k}", name=f"xt{k}")
                          for k in range(KT)]
                    for k in range(KT):
                        sf = work.tile([P, W], F16, tag="sf")
                        si = work.tile([P, W], F16, tag="si")
                        sg = work.tile([P, W], F16, tag="sg")
                        uu = work.tile([P, W], F16, tag="uu")
                        fp = scanp.tile([P, W], F16, tag=f"fp{k}", name=f"fp{k}")
                        mvv = scanp.tile([P, W], F16, tag=f"mv{k}", name=f"mv{k}")

                        def gate_mm(gate, o, n):
                            pg = psg.tile([P, 512], F32, tag="pg")
                            for kk in range(KT):
                                c0 = kk * 3 * H + gate * H + k * P
                                nc.tensor.matmul(
                                    out=pg[:, :n],
                                    lhsT=(wst[l][:, c0:c0 + P]),
                                    rhs=(xn[kk][:, o:o + n]),
                                    start=(kk == 0), stop=(kk == KT - 1))
                            return pg

                        bf = bsg[:, l * 16 + k:l * 16 + k + 1]
                        bi = bsg[:, l * 16 + 4 + k:l * 16 + 4 + k + 1]
                        bt = bsg[:, l * 16 + 8 + k:l * 16 + 8 + k + 1]
                        bth = bsg[:, l * 16 + 12 + k:l * 16 + 12 + k + 1]
                        # chunk1 (512 cols) per gate; the three 128-col
                        # chunk2 regions share one PSUM bank
                        o2 = NCH[1][0]
                        pg2 = psg.tile([P, 512], F32, tag="pg")

                        def gate_mm2(gate):
                            for kk in range(KT):
                                c0 = kk * 3 * H + gate * H + k * P
                                nc.tensor.matmul(
                                    out=pg2[:, gate * P:(gate + 1) * P],
                                    lhsT=(wst[l][:, c0:c0 + P]),
                                    rhs=(xn[kk][:, o2:]),
                                    start=(kk == 0), stop=(kk == KT - 1))

                        pg_f = gate_mm(0, 0, 512)
                        gate_mm2(0)
                        nc.scalar.activation(
                            out=sf[:, 0:512], in_=pg_f[:],
                            func=AF.Sigmoid, bias=bf)
                        nc.scalar.activation(
                            out=sf[:, o2:], in_=pg2[:, 0:P],
                            func=AF.Sigmoid, bias=bf)
                        pg_i = gate_mm(1, 0, 512)
                        gate_mm2(1)
                        nc.scalar.activation(
                            out=si[:, 0:512], in_=pg_i[:],
                            func=AF.Sigmoid, bias=bi)
                        nc.scalar.activation(
                            out=si[:, o2:], in_=pg2[:, P:2 * P],
                            func=AF.Sigmoid, bias=bi)
                        pg_t = gate_mm(2, 0, 512)
                        gate_mm2(2)
                        nc.scalar.activation(
                            out=sg[:, 0:512], in_=pg_t[:],
                            func=AF.Sigmoid, bias=bt)
                        nc.scalar.activation(
                            out=sg[:, o2:], in_=pg2[:, 2 * P:3 * P],
                            func=AF.Sigmoid, bias=bt)
                        nc.scalar.activation(
                            out=uu[:, 0:512], in_=pg_t[:],
                            func=AF.Identity, bias=bth)
                        nc.scalar.activation(
                            out=uu[:, o2:], in_=pg2[:, 2 * P:3 * P],
                            func=AF.Identity, bias=bth)
                        # full-window gate math (one pass per k)
                        ssum = work.tile([P, W], F16, tag="ssum")
                        nc.vector.tensor_add(ssum[:], sf[:], si[:])
                        rinv = work.tile([P, W], F16, tag="rinv")
                        with nc.allow_low_precision("f' in f16 is plenty"):
                            nc.vector.reciprocal(out=rinv[:], in_=ssum[:])
                        nc.vector.tensor_mul(fp[:], sf[:], rinv[:])
                        # g = max(u+0.5, sigmoid(u)); +0.5 folded into uu's
                        # bias so this is a 2x-mode f16 max
                        nc.vector.tensor_max(uu[:], uu[:], sg[:])
                        nc.vector.scalar_tensor_tensor(
                            out=mvv[:], in0=fp[:], scalar=1.0, in1=uu[:],
                            op0=OP.subtract, op1=OP.mult)
                        # boundary reset at own-region start (exact for j==0)
                        t1 = work.tile([P, 1], F32, tag="t1")
                        nc.vector.tensor_mul(
                            t1[:], fp[:, HALO:HALO + 1], rst[:, 1:2])
                        nc.vector.tensor_sub(
                            mvv[:, HALO:HALO + 1], mvv[:, HALO:HALO + 1], t1[:])
                        nc.vector.tensor_mul(
                            fp[:, HALO:HALO + 1], fp[:, HALO:HALO + 1],
                            rst[:, 0:1])
                        # h_t = f'_t h_{t-1} + v_t ;  x2 = h + x
                        nc.vector.tensor_tensor_scan(
                            out=x2[k][:], data0=fp[:], data1=mvv[:],
                            initial=0.5, op0=OP.mult, op1=OP.subtract)
                        nc.vector.tensor_add(x2[k][:], x2[k][:], x[k][:])
                    x = x2

                # ---- final LayerNorm (own tokens = groups 1..4) ----
                rows2 = ln_stats(x, 1, NG - 1, "f")
                if fc:
                    pe_filler(fc, x[0])
                rb2 = bcp.tile([P, CHUNK], F16, tag="rb2")
                mb2 = bcp.tile([P, CHUNK], F16, tag="mb2")
                for g in range(NG - 1):
                    nc.gpsimd.partition_broadcast(
                        rb2[:, g * P:(g + 1) * P],
                        rows2[0:1, g * P:(g + 1) * P])
                    nc.gpsimd.partition_broadcast(
                        mb2[:, g * P:(g + 1) * P],
                        rows2[0:1, (NG - 1 + g) * P:(NG + g) * P])
                for k in range(KT):
                    nc.vector.tensor_mul(xf_bf[k][:], x[k][:, HALO:], rb2[:])
                    nc.vector.tensor_add(xf_bf[k][:], xf_bf[k][:], mb2[:])

            # ---- phase C: logits GEMM (own 512 tokens x full vocab) ----
            # fp8e4m3 DoubleRow, 3 residual-corrected passes:
            #   po = W1@X1 + W1@X2 + W3@X1  with W1 = q8(64*w),
            #   W3 = q8(64*w - W1), X1 = q8(xf), X2 = q8(xf - X1);
            #   logits = po/64 + fc_b   (error ~1.2e-3, see prep)
            x1p = [persist.tile([P, 2, CHUNK], F8, tag=f"x1p{i}", name=f"x1p{i}")
                   for i in range(2)]
            x2p = [persist.tile([P, 2, CHUNK], F8, tag=f"x2p{i}", name=f"x2p{i}")
                   for i in range(2)]
            for k in range(KT):
                i, j = divmod(k, 2)
                nc.vector.tensor_copy(out=x1p[i][:, j, :], in_=xf_bf[k][:])
                nc.vector.tensor_sub(x2p[i][:, j, :], xf_bf[k][:],
                                     x1p[i][:, j, :])
            VG = 10   # vocab tiles per fcw load (25 groups of 10)
            DR = mybir.MatmulPerfMode.DoubleRow
            with tc.tile_pool(name="osb", bufs=8) as osbp, \
                 tc.tile_pool(name="pso", bufs=8, space="PSUM") as pso:
                for vg in range(25):
                    if prefetch:
                        fcw = fcw_tiles[vg]
                    else:
                        fcw = fcwp.tile([P, 10, 2, 2, 2, P], F8, tag="fcw")
                        nc.gpsimd.dma_start(out=fcw[:], in_=fcwt_t[vg])
                    for j in range(VG):
                        vt = vg * VG + j
                        po = pso.tile([P, CHUNK], F32, tag="po")
                        passes = [(0, x1p), (0, x2p), (1, x1p)]
                        nmm = 0
                        for (t, xs) in passes:
                            for i in range(2):
                                nc.tensor.matmul(
                                    out=po[:], lhsT=fcw[:, j, t, i, :, :],
                                    rhs=xs[i][:],
                                    start=(nmm == 0), stop=(nmm == 5),
                                    perf_mode=DR)
                                nmm += 1
                        jj = vt % 4
                        if jj == 0:
                            osb = osbp.tile([P, 2, 2, CHUNK], F16, tag="osb")
                        if jj % 2 == 0:
                            nc.scalar.activation(
                                out=osb[:, jj // 2, jj % 2, :], in_=po[:],
                                func=AF.Identity, scale=1.0 / 64,
                                bias=fcb2[:, vt:vt + 1])
                        else:
                            nc.vector.tensor_scalar(
                                out=osb[:, jj // 2, jj % 2, :], in0=po[:],
                                scalar1=1.0 / 64,
                                scalar2=fcb2[:, vt:vt + 1],
                                op0=OP.mult, op1=OP.add)
                        if jj == 3 or vt == 249:
                            b0 = (vt - jj) // 2
                            nb = (jj + 1) // 2
                            (nc.scalar if (vt // 4) % 2 == 0
                             else nc.gpsimd).dma_start(
                                out=out_t[:, b0:b0 + nb],
                                in_=osb[:, 0:nb, :, :])

    nc.compile()
    return nc


_CACHED = None


def _get_program():
    global _CACHED
    if _CACHED is None:
        _CACHED = build_program()
    return _CACHED


def prep_inputs(ids, emb, Ws, bs, ln_w, ln_b, fln_w, fc_w, fc_b):
    """Host-side layout prep -> per-core input maps."""
    ids = np.asarray(ids)
    emb = np.asarray(emb, dtype=np.float32)
    Ws = np.asarray(Ws, dtype=np.float32)
    bs = np.asarray(bs, dtype=np.float32)
    ln_w = np.asarray(ln_w, dtype=np.float32)
    ln_b = np.asarray(ln_b, dtype=np.float32)
    fln_w = np.asarray(fln_w, dtype=np.float32)
    fc_w = np.asarray(fc_w, dtype=np.float32)
    fc_b = np.asarray(fc_b, dtype=np.float32)

    emb16 = np.ascontiguousarray(emb).astype(np.float16)

    # fold ln_w into the gate weights, ln_b into the gate biases
    # Ws'[l] = Ws[l] * ln_w[l][None,:]; bias'[l] = bs[l] + Ws[l] @ ln_b[l]
    wsT = np.ascontiguousarray(
        np.stack([(Ws[l] * ln_w[l][None, :]).T.reshape(KT, P, 3 * H)
                  for l in range(L)])).astype(np.float16)
    bias = np.stack([bs[l] + Ws[l] @ ln_b[l] for l in range(L)])  # [L, 3H]

    # per-partition gate biases, grouped [l][kind][k]; kind 3 = bt + 0.5
    bsg = np.empty((P, L * 16), np.float32)
    for l in range(L):
        for gate in range(3):
            for k in range(KT):
                bsg[:, l * 16 + gate * 4 + k] = \
                    bias[l, gate * H + k * P:gate * H + (k + 1) * P]
        for k in range(KT):
            bsg[:, l * 16 + 12 + k] = \
                bias[l, 2 * H + k * P:2 * H + (k + 1) * P] + 0.5

    # fold fln_w into fc_w; quantize to fp8 e4m3 hi+residual at scale 64,
    # tiled [25, P, 10, 2(hi/res), 2(i), 2(j), P]
    import ml_dtypes
    E4 = ml_dtypes.float8_e4m3
    fcw = fc_w * fln_w[None, :]
    w1 = (64.0 * fcw).astype(E4)
    w3 = (64.0 * fcw - w1.astype(np.float32)).astype(E4)

    def _tile8(w8):
        # [H, V] -> [i 2, j 2, c P, vg 25, vt 10, m P] -> [vg, c, vt, i, j, m]
        return w8.T.reshape(2, 2, P, 25, 10, P).transpose(3, 2, 4, 0, 1, 5)

    fcwt = np.ascontiguousarray(
        np.stack([_tile8(w1), _tile8(w3)], axis=3))
    fcb2 = np.ascontiguousarray(fc_b.reshape(V // P, P).T)

    shared = {"emb": emb16, "wsT": wsT, "bsg": bsg,
              "fcwt": fcwt, "fcb": fcb2}

    in_maps = []
    for c in range(N_CORES):
        b, j = divmod(c, 4)
        own0 = j * CHUNK
        win = np.zeros(W, np.int32)
        if j == 0:
            win[HALO:] = ids[b, :CHUNK]
        else:
            win[:] = ids[b, own0 - HALO:own0 + CHUNK]
        idxt = np.ascontiguousarray(win.reshape(NG, P).T)
        rstc = np.empty((P, 2), np.float32)
        rstc[:, 0] = 0.0 if j == 0 else 1.0   # multiplies f at window pos HALO
        rstc[:, 1] = 0.5 if j == 0 else 0.0   # adds f*this to v at pos HALO
        in_maps.append({**shared, "idx": idxt, "rst": rstc})
    return in_maps


def kernel(ids, emb, Ws, bs, ln_w, ln_b, fln_w, fc_w, fc_b):
    nc = _get_program()
    in_maps = prep_inputs(ids, emb, Ws, bs, ln_w, ln_b, fln_w, fc_w, fc_b)
    res = run_bass_kernel_spmd(nc, in_maps, list(range(N_CORES)))
    out = np.empty((B, S, V), np.float32)
    for c in range(N_CORES):
        b, j = divmod(c, 4)
        arr = res.results[c]["out"]  # [P, 125, 2, CHUNK]
        out[b, j * CHUNK:(j + 1) * CHUNK, :] = \
            arr.transpose(3, 1, 2, 0).reshape(CHUNK, V).astype(np.float32)
    return out


# revision 60
# speedup vs baseline: 1.0034x; 1.0034x over previous
"""Trainium2 Bass kernel for a 3-layer minLSTM-style NLP model.

Model (per reference):
  x = emb[ids]                                   (B,S,E) = (2,2048,512)
  3 x { xn = LN(x); gates = xn @ Ws.T + bs;
        f' = sig(f)/(sig(f)+sig(i)); i' = 1-f';
        v = i' * g(tilde), g(x) = max(x+0.5, sigmoid(x));
        h_t = f'_t h_{t-1} + v_t  (h_0 = 0.5);  x = h + x }
  xf = LN(x) * fln_w;  logits = xf @ fc_w.T + fc_b    (B,S,32000)

Sharding (8 cores, zero collectives):
  core c -> (batch b=c//4, seq chunk j=c%4 of 512 tokens). Each core runs a
  640-token window (128-token halo before its own 512) through the recurrent
  stack; the forget-product decays the unknown initial state to ~0 over the
  halo, and a per-core reset constant makes j==0 exact at the batch start.
  Each core computes logits for its own 512 tokens against the full vocab.

Key layout tricks:
  - LN affine (ln_w/ln_b) folded into the gate weights/biases on host, and
    fln_w folded into fc_w, so the device only applies (x-m)*rstd.
  - Per-token LN stats computed TRANSPOSED (tokens on partitions) via 1-row
    matmuls; rsqrt via magic-constant Newton on tiny [128,5] tiles; the
    (rstd, -m*rstd) stats transposed back in ONE PE transpose and broadcast
    across partitions by the Pool engine.
  - Mean correction applied as xn = x*rstd + bcast(-m*rstd) on DVE (no
    rank-1 GEMM update).
  - Gate math: f' = sf/(sf+si) via a single DVE divide; v-side fused with
    scalar_tensor_tensor so the scan consumes (f'-1)*g with op1=subtract.
  - fc_w streamed as fp8 e4m3 hi+residual (3-pass residual-corrected GEMM in
    DoubleRow mode); the 32.8MB weight stream is prefetched on a dedicated
    DMA queue starting at t=0 so phase C stays PE-bound.
  - Activations f16 everywhere; output logits written f16 and upcast on host.
"""

import sys

if "/opt/trn_rl_repo" not in sys.path:
    sys.path.insert(0, "/opt/trn_rl_repo")

import numpy as np

import concourse.bass as bass
import concourse.bacc as bacc
import concourse.tile as tile
from concourse import mybir
from concourse.bass import IndirectOffsetOnAxis
from concourse.bass_utils import run_bass_kernel_spmd
from concourse.masks import make_identity

F32 = mybir.dt.float32
F16 = mybir.dt.float16
F8 = mybir.dt.float8e4
I32 = mybir.dt.int32
AF = mybir.ActivationFunctionType
OP = mybir.AluOpType

# problem constants
B, S, V, H, L = 2, 2048, 32000, 512, 3
P = 128
KT = H // P            # 4 k-tiles over the H contraction dim
CHUNK = 512            # own tokens per core
HALO = 128             # speculative scan warmup tokens
W = HALO + CHUNK       # 640 window tokens per core
NG = W // P            # 5 embedding gather groups
NCH = [(0, 512), (512, 128)]   # window free-dim chunks (PSUM-bounded)
N_CORES = 8
EPS = 1e-5
MAGIC2 = 0x1EF759DF    # rsqrt seed magic, pre-adjusted for hneg=-(var+eps)/2


def build_program(fcwb=11, psgb=3, pstb=3, workb=2, fa=8, fb=4, fc=12, u_eng="act", prefetch=True):
    nc = bacc.Bacc("TRN2", target_bir_lowering=False, debug=False,
                   enable_asserts=True, num_devices=N_CORES)

    idx_t = nc.dram_tensor("idx", [P, NG], I32, kind="ExternalInput").ap()
    emb_t = nc.dram_tensor("emb", [V, H], F16, kind="ExternalInput").ap()
    wsT_t = nc.dram_tensor("wsT", [L, KT, P, 3 * H], F16, kind="ExternalInput").ap()
    bsg_t = nc.dram_tensor("bsg", [P, L * 16], F32, kind="ExternalInput").ap()
    fcwt_t = nc.dram_tensor("fcwt", [25, P, 10, 2, 2, 2, P], F8, kind="ExternalInput").ap()
    fcb_t = nc.dram_tensor("fcb", [P, V // P], F32, kind="ExternalInput").ap()
    rst_t = nc.dram_tensor("rst", [P, 2], F32, kind="ExternalInput").ap()
    out_t = nc.dram_tensor("out", [P, V // (2 * P), 2, CHUNK], F16,
                           kind="ExternalOutput").ap()

    with tile.TileContext(nc) as tc:
        with tc.tile_pool(name="singles", bufs=1) as singles, \
             tc.tile_pool(name="persist", bufs=1) as persist, \
             tc.tile_pool(name="wst", bufs=2) as wstp, \
             tc.tile_pool(name="fcw", bufs=fcwb) as fcwp:

            # ---- constants / small inputs (scalar queue: fast dispatch) ----
            idx = singles.tile([P, NG], I32)
            nc.scalar.dma_start(out=idx[:], in_=idx_t[:])
            bsg = singles.tile([P, L * 16], F32)
            nc.scalar.dma_start(out=bsg[:], in_=bsg_t[:])
            rst = singles.tile([P, 2], F32)
            nc.scalar.dma_start(out=rst[:], in_=rst_t[:])
            fcb2 = singles.tile([P, V // P], F32)
            nc.scalar.dma_start(out=fcb2[:], in_=fcb_t[:])
            ident16 = singles.tile([P, P], F16)
            make_identity(nc, ident16[:])
            actwarm = singles.tile([1, 1], F32)
            nc.scalar.activation(out=actwarm[:], in_=rst[0:1, 0:1],
                                 func=AF.Sigmoid)
            ones16 = singles.tile([P, 1], F16)   # stats-reduce rhs
            nc.vector.memset(ones16[:], 1.0)

            # ---- gate weights: 2-deep rotation; l0/l1 up front (sync queue,
            # ahead of the fcw stream), l2 JIT on the Pool queue during l0 ----
            def load_wst(l, queue):
                w = wstp.tile([P, KT * 3 * H], F16, tag="wst", name=f"wst{l}")
                for kk in range(KT):
                    queue.dma_start(
                        out=w[:, kk * 3 * H:(kk + 1) * 3 * H],
                        in_=wsT_t[l, kk])
                return w

            wst = [load_wst(0, nc.sync), load_wst(1, nc.sync), None]

            # ---- fc_w fp8 stream: issue every load now; the pool's buf
            # rotation throttles against phase C's consumption ----
            fcw_tiles = []
            if prefetch:
                for vg in range(25):
                    fcw = fcwp.tile([P, 10, 2, 2, 2, P], F8, tag="fcw")
                    nc.sync.dma_start(out=fcw[:], in_=fcwt_t[vg])
                    fcw_tiles.append(fcw)

            # final activations (channel-major), consumed by phase C
            xf_bf = [persist.tile([P, CHUNK], F16, tag=f"xfbf{k}", name=f"xfbf{k}")
                     for k in range(KT)]

            with tc.tile_pool(name="xpool", bufs=2) as xpool, \
                 tc.tile_pool(name="work", bufs=workb) as work, \
                 tc.tile_pool(name="scan", bufs=1) as scanp, \
                 tc.tile_pool(name="xnp", bufs=1) as xnp, \
                 tc.tile_pool(name="bc", bufs=1) as bcp, \
                 tc.tile_pool(name="stat", bufs=1) as statp, \
                 tc.tile_pool(name="psg", bufs=psgb, space="PSUM") as psg, \
                 tc.tile_pool(name="pss", bufs=1, space="PSUM") as pss, \
                 tc.tile_pool(name="psgr", bufs=1, space="PSUM") as psgr, \
                 tc.tile_pool(name="pst", bufs=pstb, space="PSUM") as pst:

                # ---- phase A: embedding gather + transpose to channel-major
                x = [xpool.tile([P, W], F16, tag=f"x{k}", name=f"xt{k}")
                     for k in range(KT)]
                with tc.tile_pool(name="gath", bufs=1) as gathp:
                    xgs = []
                    for g in range(NG):
                        xg = gathp.tile([P, H], F16, tag=f"xg{g}", name=f"xg{g}")
                        nc.gpsimd.indirect_dma_start(
                            out=xg[:], out_offset=None, in_=emb_t[:],
                            in_offset=IndirectOffsetOnAxis(ap=idx[:, g:g + 1], axis=0),
                        )
                        xgs.append(xg)
                    for g in range(NG):
                        xg = xgs[g]
                        for k in range(KT):
                            ptr = pst.tile([P, P], F16, tag="pstt", name="ptr")
                            nc.tensor.transpose(
                                out=ptr[:], in_=xg[:, k * P:(k + 1) * P],
                                identity=ident16[:])
                            eng = nc.vector if (g * KT + k) % 2 == 0 else nc.scalar
                            if eng is nc.vector:
                                nc.vector.tensor_copy(
                                    out=x[k][:, g * P:(g + 1) * P], in_=ptr[:])
                            else:
                                nc.scalar.copy(
                                    out=x[k][:, g * P:(g + 1) * P], in_=ptr[:])

                # ---- helper: transposed LN stats + rsqrt newton ----
                def ln_stats(xs, g0, ngr, tag):
                    """Per-token rstd and -mean*rstd for token groups
                    [g0, g0+ngr): returned as a [2*ngr, P] f16 tile whose row
                    q holds rstd (q < ngr) / -m*rstd (q >= ngr) for group q."""
                    psT = pss.tile([P, 8], F32, tag="psT", name="psT")
                    sums_b = statp.tile([P, ngr], F32, tag="sumb")
                    scr = statp.tile([P, P], F32, tag="ttrscr")
                    # 4 gram slots in one PSUM bank so group g+1's matmuls
                    # overlap group g's diag extraction
                    gram = psgr.tile([P, 4 * P], F32, tag="gram", name="gram")
                    for g in range(ngr):
                        sl = slice((g0 + g) * P, (g0 + g + 1) * P)
                        gsl = slice((g % 4) * P, (g % 4 + 1) * P)
                        for k in range(KT):
                            nc.tensor.matmul(
                                out=psT[:, g:g + 1], lhsT=xs[k][:, sl],
                                rhs=ones16[:],
                                start=(k == 0), stop=(k == KT - 1))
                        for k in range(KT):
                            nc.tensor.matmul(
                                out=gram[:, gsl], lhsT=xs[k][:, sl],
                                rhs=xs[k][:, sl],
                                start=(k == 0), stop=(k == KT - 1))
                        # sum_x2 = diag(gram) via (gram * I) row-reduce
                        nc.vector.tensor_tensor(
                            out=scr[:], in0=gram[:, gsl], in1=ident16[:],
                            op=OP.mult)
                        nc.vector.tensor_reduce(
                            out=sums_b[:, g:g + 1], in_=scr[:], op=OP.add,
                            axis=mybir.AxisListType.X)
                    m2 = statp.tile([P, ngr], F32, tag="m2")
                    hneg = statp.tile([P, ngr], F32, tag="hneg")
                    y = statp.tile([P, ngr], F32, tag="y")
                    t = statp.tile([P, ngr], F32, tag="t")
                    rsmr = statp.tile([P, 2 * ngr], F16, tag="rsmr")
                    # m2 = sum_x^2/(2 H^2) = m^2/2, all-DVE: stage the PSUM
                    # sums into SBUF first (s2s2d2 forbids two PSUM reads)
                    psTs = statp.tile([P, ngr], F32, tag="psTs")
                    nc.vector.tensor_copy(out=psTs[:], in_=psT[:, 0:ngr])
                    nc.vector.scalar_tensor_tensor(
                        out=m2[:], in0=psTs[:], scalar=0.5 / (H * H),
                        in1=psTs[:], op0=OP.mult, op1=OP.mult)
                    # hneg = m^2/2 - (sum_x2/(2H) + eps/2) = -(var+eps)/2
                    nc.vector.tensor_scalar(
                        out=hneg[:], in0=sums_b[:],
                        scalar1=0.5 / H, scalar2=EPS / 2,
                        op0=OP.mult, op1=OP.add)
                    nc.vector.tensor_sub(hneg[:], m2[:], hneg[:])
                    # rsqrt seed: y = -( (bits(hneg)>>1) - MAGIC2 )
                    nc.vector.tensor_scalar(
                        out=y[:].bitcast(I32), in0=hneg[:].bitcast(I32),
                        scalar1=1, scalar2=None,
                        op0=OP.arith_shift_right)
                    nc.vector.tensor_scalar(
                        out=y[:].bitcast(I32), in0=y[:].bitcast(I32),
                        scalar1=MAGIC2, scalar2=-1, op0=OP.subtract,
                        op1=OP.mult)
                    nc.vector.tensor_mul(t[:], y[:], y[:])
                    nc.vector.tensor_mul(t[:], t[:], hneg[:])
                    nc.vector.scalar_tensor_tensor(
                        out=y[:], in0=t[:], scalar=1.5, in1=y[:],
                        op0=OP.add, op1=OP.mult)
                    nc.vector.tensor_copy(out=rsmr[:, 0:ngr], in_=y[:])
                    # mr = -(sum_x/H)*rstd
                    nc.vector.scalar_tensor_tensor(
                        out=rsmr[:, ngr:2 * ngr], in0=psT[:, 0:ngr],
                        scalar=-1.0 / H, in1=y[:], op0=OP.mult, op1=OP.mult)
                    # transpose each column separately so every row lands
                    # on partition 0 (partition_broadcast requirement)
                    rows = statp.tile([1, 2 * NG * P], F16, tag=f"rows{tag}",
                                      name=f"rows{tag}")
                    for q in range(2 * ngr):
                        ptrq = pst.tile([1, P], F16, tag="pstt", name="ptrq")
                        nc.tensor.transpose(out=ptrq[:], in_=rsmr[:, q:q + 1],
                                            identity=ident16[:])
                        nc.scalar.copy(
                            out=rows[0:1, q * P:(q + 1) * P], in_=ptrq[:])
                    return rows

                def pe_filler(n, rhs_tile):
                    """Keep the PE p-state ramp warm with dead matmuls into
                    a rotating PSUM tile nobody reads."""
                    for _ in range(n):
                        pg = psg.tile([P, 512], F32, tag="pg", name="fill")
                        nc.tensor.matmul(out=pg[:], lhsT=ident16[:],
                                         rhs=rhs_tile[:, 0:512],
                                         start=True, stop=True)

                # ---- phase B: L recurrent layers ----
                for l in range(L):
                    if l == 0:
                        wst[2] = load_wst(2, nc.gpsimd)
                    rows = ln_stats(x, 0, NG, "b")
                    if fa:
                        pe_filler(fa, x[0])

                    # broadcast rstd / -m*rstd across partitions (Pool);
                    # mrb broadcasts overlap the xn multiplies
                    rb = bcp.tile([P, W], F16, tag="rb")
                    mrb = bcp.tile([P, W], F16, tag="mrb")
                    for g in range(NG):
                        nc.gpsimd.partition_broadcast(
                            rb[:, g * P:(g + 1) * P],
                            rows[0:1, g * P:(g + 1) * P])
                    # xn = x*rstd - m*rstd
                    xn = [xnp.tile([P, W], F16, tag=f"xn{k}", name=f"xn{k}")
                          for k in range(KT)]
                    for k in range(KT):
                        nc.vector.tensor_mul(xn[k][:], x[k][:], rb[:])
                    for g in range(NG):
                        nc.gpsimd.partition_broadcast(
                            mrb[:, g * P:(g + 1) * P],
                            rows[0:1, (NG + g) * P:(NG + g + 1) * P])
                    for k in range(KT):
                        nc.vector.tensor_add(xn[k][:], xn[k][:], mrb[:])
                    if fb:
                        pe_filler(fb, x[0])

                    # --- gates GEMM + nonlinearities + scan ---
                    x2 = [xpool.tile([P, W], F16, tag=f"x{k}", name=f"xt{k}")
                          for k in range(KT)]
                    for k in range(KT):
                        sf = work.tile([P, W], F16, tag="sf")
                        si = work.tile([P, W], F16, tag="si")
                        sg = work.tile([P, W], F16, tag="sg")
                        uu = work.tile([P, W], F16, tag="uu")
                        fp = scanp.tile([P, W], F16, tag=f"fp{k}", name=f"fp{k}")
                        mvv = scanp.tile([P, W], F16, tag=f"mv{k}", name=f"mv{k}")

                        def gate_mm(gate, o, n):
                            pg = psg.tile([P, 512], F32, tag="pg")
                            for kk in range(KT):
                                c0 = kk * 3 * H + gate * H + k * P
                                nc.tensor.matmul(
                                    out=pg[:, :n],
                                    lhsT=(wst[l][:, c0:c0 + P]),
                                    rhs=(xn[kk][:, o:o + n]),
                                    start=(kk == 0), stop=(kk == KT - 1))
                            return pg

                        bf = bsg[:, l * 16 + k:l * 16 + k + 1]
                        bi = bsg[:, l * 16 + 4 + k:l * 16 + 4 + k + 1]
                        bt = bsg[:, l * 16 + 8 + k:l * 16 + 8 + k + 1]
                        bth = bsg[:, l * 16 + 12 + k:l * 16 + 12 + k + 1]
                        # chunk1 (512 cols) per gate; the three 128-col
                        # chunk2 regions share one PSUM bank
                        o2 = NCH[1][0]
                        pg2 = psg.tile([P, 512], F32, tag="pg")

                        def gate_mm2(gate):
                            for kk in range(KT):
                                c0 = kk * 3 * H + gate * H + k * P
                                nc.tensor.matmul(
                                    out=pg2[:, gate * P:(gate + 1) * P],
                                    lhsT=(wst[l][:, c0:c0 + P]),
                                    rhs=(xn[kk][:, o2:]),
                                    start=(kk == 0), stop=(kk == KT - 1))

                        pg_f = gate_mm(0, 0, 512)
                        gate_mm2(0)
                        nc.scalar.activation(
                            out=sf[:, 0:512], in_=pg_f[:],
                            func=AF.Sigmoid, bias=bf)
                        nc.scalar.activation(
                            out=sf[:, o2:], in_=pg2[:, 0:P],
                            func=AF.Sigmoid, bias=bf)
                        pg_i = gate_mm(1, 0, 512)
                        gate_mm2(1)
                        nc.scalar.activation(
                            out=si[:, 0:512], in_=pg_i[:],
                            func=AF.Sigmoid, bias=bi)
                        nc.scalar.activation(
                            out=si[:, o2:], in_=pg2[:, P:2 * P],
                            func=AF.Sigmoid, bias=bi)
                        pg_t = gate_mm(2, 0, 512)
                        gate_mm2(2)
                        nc.scalar.activation(
                            out=sg[:, 0:512], in_=pg_t[:],
                            func=AF.Sigmoid, bias=bt)
                        nc.scalar.activation(
                            out=sg[:, o2:], in_=pg2[:, 2 * P:3 * P],
                            func=AF.Sigmoid, bias=bt)
                        nc.scalar.activation(
                            out=uu[:, 0:512], in_=pg_t[:],
                            func=AF.Identity, bias=bth)
                        nc.scalar.activation(
                            out=uu[:, o2:], in_=pg2[:, 2 * P:3 * P],
                            func=AF.Identity, bias=bth)
                        # full-window gate math (one pass per k)
                        ssum = work.tile([P, W], F16, tag="ssum")
                        nc.vector.tensor_add(ssum[:], sf[:], si[:])
                        rinv = work.tile([P, W], F16, tag="rinv")
                        with nc.allow_low_precision("f' in f16 is plenty"):
                            nc.vector.reciprocal(out=rinv[:], in_=ssum[:])
                        nc.vector.tensor_mul(fp[:], sf[:], rinv[:])
                        # g = max(u+0.5, sigmoid(u)); +0.5 folded into uu's
                        # bias so this is a 2x-mode f16 max
                        nc.vector.tensor_max(uu[:], uu[:], sg[:])
                        nc.vector.scalar_tensor_tensor(
                            out=mvv[:], in0=fp[:], scalar=1.0, in1=uu[:],
                            op0=OP.subtract, op1=OP.mult)
                        # boundary reset at own-region start (exact for j==0)
                        t1 = work.tile([P, 1], F32, tag="t1")
                        nc.vector.tensor_mul(
                            t1[:], fp[:, HALO:HALO + 1], rst[:, 1:2])
                        nc.vector.tensor_sub(
                            mvv[:, HALO:HALO + 1], mvv[:, HALO:HALO + 1], t1[:])
                        nc.vector.tensor_mul(
                            fp[:, HALO:HALO + 1], fp[:, HALO:HALO + 1],
                            rst[:, 0:1])
                        # h_t = f'_t h_{t-1} + v_t ;  x2 = h + x
                        nc.vector.tensor_tensor_scan(
                            out=x2[k][:], data0=fp[:], data1=mvv[:],
                            initial=0.5, op0=OP.mult, op1=OP.subtract)
                        nc.vector.tensor_add(x2[k][:], x2[k][:], x[k][:])
                    x = x2

                # ---- final LayerNorm (own tokens = groups 1..4) ----
                rows2 = ln_stats(x, 1, NG - 1, "f")
                if fc:
                    pe_filler(fc, x[0])
                rb2 = bcp.tile([P, CHUNK], F16, tag="rb2")
                mb2 = bcp.tile([P, CHUNK], F16, tag="mb2")
                for g in range(NG - 1):
                    nc.gpsimd.partition_broadcast(
                        rb2[:, g * P:(g + 1) * P],
                        rows2[0:1, g * P:(g + 1) * P])
                    nc.gpsimd.partition_broadcast(
                        mb2[:, g * P:(g + 1) * P],
                        rows2[0:1, (NG - 1 + g) * P:(NG + g) * P])
                for k in range(KT):
                    nc.vector.tensor_mul(xf_bf[k][:], x[k][:, HALO:], rb2[:])
                    nc.vector.tensor_add(xf_bf[k][:], xf_bf[k][:], mb2[:])

            # ---- phase C: logits GEMM (own 512 tokens x full vocab) ----
            # fp8e4m3 DoubleRow, 3 residual-corrected passes:
            #   po = W1@X1 + W1@X2 + W3@X1  with W1 = q8(64*w),
            #   W3 = q8(64*w - W1), X1 = q8(xf), X2 = q8(xf - X1);
            #   logits = po/64 + fc_b   (error ~1.2e-3, see prep)
            x1p = [persist.tile([P, 2, CHUNK], F8, tag=f"x1p{i}", name=f"x1p{i}")
                   for i in range(2)]
            x2p = [persist.tile([P, 2, CHUNK], F8, tag=f"x2p{i}", name=f"x2p{i}")
                   for i in range(2)]
            for k in range(KT):
                i, j = divmod(k, 2)
                nc.vector.tensor_copy(out=x1p[i][:, j, :], in_=xf_bf[k][:])
            for k in range(KT):
                i, j = divmod(k, 2)
                nc.vector.tensor_sub(x2p[i][:, j, :], xf_bf[k][:],
                                     x1p[i][:, j, :])
            VG = 10   # vocab tiles per fcw load (25 groups of 10)
            DR = mybir.MatmulPerfMode.DoubleRow
            with tc.tile_pool(name="osb", bufs=8) as osbp, \
                 tc.tile_pool(name="pso", bufs=8, space="PSUM") as pso:
                for vg in range(25):
                    if prefetch:
                        fcw = fcw_tiles[vg]
                    else:
                        fcw = fcwp.tile([P, 10, 2, 2, 2, P], F8, tag="fcw")
                        nc.gpsimd.dma_start(out=fcw[:], in_=fcwt_t[vg])
                    for j in range(VG):
                        vt = vg * VG + j
                        po = pso.tile([P, CHUNK], F32, tag="po")
                        passes = [(0, x1p), (1, x1p), (0, x2p)]
                        nmm = 0
                        for (t, xs) in passes:
                            for i in range(2):
                                nc.tensor.matmul(
                                    out=po[:], lhsT=fcw[:, j, t, i, :, :],
                                    rhs=xs[i][:],
                                    start=(nmm == 0), stop=(nmm == 5),
                                    perf_mode=DR)
                                nmm += 1
                        jj = vt % 4
                        if jj == 0:
                            osb = osbp.tile([P, 2, 2, CHUNK], F16, tag="osb")
                        if jj % 2 == 0:
                            nc.scalar.activation(
                                out=osb[:, jj // 2, jj % 2, :], in_=po[:],
                                func=AF.Identity, scale=1.0 / 64,
                                bias=fcb2[:, vt:vt + 1])
                        else:
                            nc.vector.tensor_scalar(
                                out=osb[:, jj // 2, jj % 2, :], in0=po[:],
                                scalar1=1.0 / 64,
                                scalar2=fcb2[:, vt:vt + 1],
                                op0=OP.mult, op1=OP.add)
                        if jj == 3 or vt == 249:
                            b0 = (vt - jj) // 2
                            nb = (jj + 1) // 2
                            (nc.scalar if (vt // 4) % 2 == 0
                             else nc.gpsimd).dma_start(
                                out=out_t[:, b0:b0 + nb],
                                in_=osb[:, 0:nb, :, :])

    nc.compile()
    return nc


_CACHED = None


def _get_program():
    global _CACHED
    if _CACHED is None:
        _CACHED = build_program()
    return _CACHED


def prep_inputs(ids, emb, Ws, bs, ln_w, ln_b, fln_w, fc_w, fc_b):
    """Host-side layout prep -> per-core input maps."""
    ids = np.asarray(ids)
    emb = np.asarray(emb, dtype=np.float32)
    Ws = np.asarray(Ws, dtype=np.float32)
    bs = np.asarray(bs, dtype=np.float32)
    ln_w = np.asarray(ln_w, dtype=np.float32)
    ln_b = np.asarray(ln_b, dtype=np.float32)
    fln_w = np.asarray(fln_w, dtype=np.float32)
    fc_w = np.asarray(fc_w, dtype=np.float32)
    fc_b = np.asarray(fc_b, dtype=np.float32)

    emb16 = np.ascontiguousarray(emb).astype(np.float16)

    # fold ln_w into the gate weights, ln_b into the gate biases
    # Ws'[l] = Ws[l] * ln_w[l][None,:]; bias'[l] = bs[l] + Ws[l] @ ln_b[l]
    wsT = np.ascontiguousarray(
        np.stack([(Ws[l] * ln_w[l][None, :]).T.reshape(KT, P, 3 * H)
                  for l in range(L)])).astype(np.float16)
    bias = np.stack([bs[l] + Ws[l] @ ln_b[l] for l in range(L)])  # [L, 3H]

    # per-partition gate biases, grouped [l][kind][k]; kind 3 = bt + 0.5
    bsg = np.empty((P, L * 16), np.float32)
    for l in range(L):
        for gate in range(3):
            for k in range(KT):
                bsg[:, l * 16 + gate * 4 + k] = \
                    bias[l, gate * H + k * P:gate * H + (k + 1) * P]
        for k in range(KT):
            bsg[:, l * 16 + 12 + k] = \
                bias[l, 2 * H + k * P:2 * H + (k + 1) * P] + 0.5

    # fold fln_w into fc_w; quantize to fp8 e4m3 hi+residual at scale 64,
    # tiled [25, P, 10, 2(hi/res), 2(i), 2(j), P]
    import ml_dtypes
    E4 = ml_dtypes.float8_e4m3
    fcw = fc_w * fln_w[None, :]
    w1 = (64.0 * fcw).astype(E4)
    w3 = (64.0 * fcw - w1.astype(np.float32)).astype(E4)

    def _tile8(w8):
        # [H, V] -> [i 2, j 2, c P, vg 25, vt 10, m P] -> [vg, c, vt, i, j, m]
        return w8.T.reshape(2, 2, P, 25, 10, P).transpose(3, 2, 4, 0, 1, 5)

    fcwt = np.ascontiguousarray(
        np.stack([_tile8(w1), _tile8(w3)], axis=3))
    fcb2 = np.ascontiguousarray(fc_b.reshape(V // P, P).T)

    shared = {"emb": emb16, "wsT": wsT, "bsg": bsg,
              "fcwt": fcwt, "fcb": fcb2}

    in_maps = []
    for c in range(N_CORES):
        b, j = divmod(c, 4)
        own0 = j * CHUNK
        win = np.zeros(W, np.int32)
        if j == 0:
            win[HALO:] = ids[b, :CHUNK]
        else:
            win[:] = ids[b, own0 - HALO:own0 + CHUNK]
        idxt = np.ascontiguousarray(win.reshape(NG, P).T)
        rstc = np.empty((P, 2), np.float32)
        rstc[:, 0] = 0.0 if j == 0 else 1.0   # multiplies f at window pos HALO
        rstc[:, 1] = 0.5 if j == 0 else 0.0   # adds f*this to v at pos HALO
        in_maps.append({**shared, "idx": idxt, "rst": rstc})
    return in_maps


def kernel(ids, emb, Ws, bs, ln_w, ln_b, fln_w, fc_w, fc_b):
    nc = _get_program()
    in_maps = prep_inputs(ids, emb, Ws, bs, ln_w, ln_b, fln_w, fc_w, fc_b)
    res = run_bass_kernel_spmd(nc, in_maps, list(range(N_CORES)))
    out = np.empty((B, S, V), np.float32)
    for c in range(N_CORES):
        b, j = divmod(c, 4)
        arr = res.results[c]["out"]  # [P, 125, 2, CHUNK]
        out[b, j * CHUNK:(j + 1) * CHUNK, :] = \
            arr.transpose(3, 1, 2, 0).reshape(CHUNK, V).astype(np.float32)
    return out


# revision 62
# speedup vs baseline: 1.0296x; 1.0261x over previous
"""Trainium2 Bass kernel for a 3-layer minLSTM-style NLP model.

Model (per reference):
  x = emb[ids]                                   (B,S,E) = (2,2048,512)
  3 x { xn = LN(x); gates = xn @ Ws.T + bs;
        f' = sig(f)/(sig(f)+sig(i)); i' = 1-f';
        v = i' * g(tilde), g(x) = max(x+0.5, sigmoid(x));
        h_t = f'_t h_{t-1} + v_t  (h_0 = 0.5);  x = h + x }
  xf = LN(x) * fln_w;  logits = xf @ fc_w.T + fc_b    (B,S,32000)

Sharding (8 cores, zero collectives):
  core c -> (batch b=c//4, seq chunk j=c%4 of 512 tokens). Each core runs a
  640-token window (128-token halo before its own 512) through the recurrent
  stack; the forget-product decays the unknown initial state to ~0 over the
  halo, and a per-core reset constant makes j==0 exact at the batch start.
  Each core computes logits for its own 512 tokens against the full vocab.

Key layout tricks:
  - LN affine (ln_w/ln_b) folded into the gate weights/biases on host, and
    fln_w folded into fc_w, so the device only applies (x-m)*rstd.
  - Per-token LN stats computed TRANSPOSED (tokens on partitions) via 1-row
    matmuls; rsqrt via magic-constant Newton on tiny [128,5] tiles; the
    (rstd, -m*rstd) stats transposed back in ONE PE transpose and broadcast
    across partitions by the Pool engine.
  - Mean correction applied as xn = x*rstd + bcast(-m*rstd) on DVE (no
    rank-1 GEMM update).
  - Gate math: f' = sf/(sf+si) via a single DVE divide; v-side fused with
    scalar_tensor_tensor so the scan consumes (f'-1)*g with op1=subtract.
  - fc_w streamed as fp8 e4m3 hi+residual (3-pass residual-corrected GEMM in
    DoubleRow mode); the 32.8MB weight stream is prefetched on a dedicated
    DMA queue starting at t=0 so phase C stays PE-bound.
  - Activations f16 everywhere; output logits written f16 and upcast on host.
"""

import sys

if "/opt/trn_rl_repo" not in sys.path:
    sys.path.insert(0, "/opt/trn_rl_repo")

import numpy as np

import concourse.bass as bass
import concourse.bacc as bacc
import concourse.tile as tile
from concourse import mybir
from concourse.bass import IndirectOffsetOnAxis
from concourse.bass_utils import run_bass_kernel_spmd
from concourse.masks import make_identity

F32 = mybir.dt.float32
F16 = mybir.dt.float16
F8 = mybir.dt.float8e4
I32 = mybir.dt.int32
AF = mybir.ActivationFunctionType
OP = mybir.AluOpType

# problem constants
B, S, V, H, L = 2, 2048, 32000, 512, 3
P = 128
KT = H // P            # 4 k-tiles over the H contraction dim
CHUNK = 512            # own tokens per core
HALO = 128             # speculative scan warmup tokens
W = HALO + CHUNK       # 640 window tokens per core
NG = W // P            # 5 embedding gather groups
NCH = [(0, 512), (512, 128)]   # window free-dim chunks (PSUM-bounded)
N_CORES = 8
EPS = 1e-5
MAGIC2 = 0x1EF759DF    # rsqrt seed magic, pre-adjusted for hneg=-(var+eps)/2


def build_program(fcwb=11, psgb=3, pstb=3, workb=2, fa=8, fb=4, fc=12, u_eng="act", prefetch=True):
    nc = bacc.Bacc("TRN2", target_bir_lowering=False, debug=False,
                   enable_asserts=True, num_devices=N_CORES)

    idx_t = nc.dram_tensor("idx", [P, NG], I32, kind="ExternalInput").ap()
    emb_t = nc.dram_tensor("emb", [V, H], F16, kind="ExternalInput").ap()
    wsT_t = nc.dram_tensor("wsT", [L, KT, P, 3 * H], F16, kind="ExternalInput").ap()
    bsg_t = nc.dram_tensor("bsg", [P, L * 16], F32, kind="ExternalInput").ap()
    fcwt_t = nc.dram_tensor("fcwt", [25, P, 10, 2, 2, 2, P], F8, kind="ExternalInput").ap()
    fcb_t = nc.dram_tensor("fcb", [P, V // P], F32, kind="ExternalInput").ap()
    rst_t = nc.dram_tensor("rst", [P, 2], F32, kind="ExternalInput").ap()
    out_t = nc.dram_tensor("out", [P, V // (2 * P), 2, CHUNK], F16,
                           kind="ExternalOutput").ap()

    with tile.TileContext(nc) as tc:
        with tc.tile_pool(name="singles", bufs=1) as singles, \
             tc.tile_pool(name="persist", bufs=1) as persist, \
             tc.tile_pool(name="wst", bufs=2) as wstp, \
             tc.tile_pool(name="fcw", bufs=fcwb) as fcwp:

            # ---- constants / small inputs (scalar queue: fast dispatch) ----
            idx = singles.tile([P, NG], I32)
            nc.scalar.dma_start(out=idx[:], in_=idx_t[:])
            bsg = singles.tile([P, L * 16], F32)
            nc.scalar.dma_start(out=bsg[:], in_=bsg_t[:])
            rst = singles.tile([P, 2], F32)
            nc.scalar.dma_start(out=rst[:], in_=rst_t[:])
            fcb2 = singles.tile([P, V // P], F32)
            nc.scalar.dma_start(out=fcb2[:], in_=fcb_t[:])
            ident16 = singles.tile([P, P], F16)
            make_identity(nc, ident16[:])
            actwarm = singles.tile([1, 1], F32)
            nc.scalar.activation(out=actwarm[:], in_=rst[0:1, 0:1],
                                 func=AF.Sigmoid)
            ones16 = singles.tile([P, 1], F16)   # stats-reduce rhs
            nc.vector.memset(ones16[:], 1.0)

            # ---- gate weights: 2-deep rotation; l0/l1 up front (sync queue,
            # ahead of the fcw stream), l2 JIT on the Pool queue during l0 ----
            def load_wst(l, queue):
                w = wstp.tile([P, KT * 3 * H], F16, tag="wst", name=f"wst{l}")
                for kk in range(KT):
                    queue.dma_start(
                        out=w[:, kk * 3 * H:(kk + 1) * 3 * H],
                        in_=wsT_t[l, kk])
                return w

            wst = [load_wst(0, nc.sync), load_wst(1, nc.sync), None]

            # ---- fc_w fp8 stream: issue every load now; the pool's buf
            # rotation throttles against phase C's consumption ----
            fcw_tiles = []
            if prefetch:
                for vg in range(25):
                    fcw = fcwp.tile([P, 10, 2, 2, 2, P], F8, tag="fcw")
                    nc.sync.dma_start(out=fcw[:], in_=fcwt_t[vg])
                    fcw_tiles.append(fcw)

            # final activations (channel-major), consumed by phase C
            xf_bf = [persist.tile([P, CHUNK], F16, tag=f"xfbf{k}", name=f"xfbf{k}")
                     for k in range(KT)]

            with tc.tile_pool(name="xpool", bufs=2) as xpool, \
                 tc.tile_pool(name="work", bufs=workb) as work, \
                 tc.tile_pool(name="scan", bufs=1) as scanp, \
                 tc.tile_pool(name="xnp", bufs=1) as xnp, \
                 tc.tile_pool(name="bc", bufs=1) as bcp, \
                 tc.tile_pool(name="stat", bufs=1) as statp, \
                 tc.tile_pool(name="psg", bufs=psgb, space="PSUM") as psg, \
                 tc.tile_pool(name="pss", bufs=1, space="PSUM") as pss, \
                 tc.tile_pool(name="psgr", bufs=1, space="PSUM") as psgr, \
                 tc.tile_pool(name="pst", bufs=pstb, space="PSUM") as pst:

                # ---- phase A: embedding gather + transpose to channel-major
                x = [xpool.tile([P, W], F16, tag=f"x{k}", name=f"xt{k}")
                     for k in range(KT)]
                with tc.tile_pool(name="gath", bufs=1) as gathp:
                    # one gather for all NG groups: a single SWDGE generation
                    # pass instead of five serial ~1us ones
                    xgall = gathp.tile([P, NG, H], F16, tag="xgall",
                                       name="xgall")
                    nc.gpsimd.indirect_dma_start(
                        out=xgall[:], out_offset=None, in_=emb_t[:],
                        in_offset=IndirectOffsetOnAxis(ap=idx[:, 0:NG], axis=0),
                    )
                    for g in range(NG):
                        for k in range(KT):
                            ptr = pst.tile([P, P], F16, tag="pstt", name="ptr")
                            nc.tensor.transpose(
                                out=ptr[:],
                                in_=xgall[:, g, k * P:(k + 1) * P],
                                identity=ident16[:])
                            eng = nc.vector if (g * KT + k) % 2 == 0 else nc.scalar
                            if eng is nc.vector:
                                nc.vector.tensor_copy(
                                    out=x[k][:, g * P:(g + 1) * P], in_=ptr[:])
                            else:
                                nc.scalar.copy(
                                    out=x[k][:, g * P:(g + 1) * P], in_=ptr[:])

                # ---- helper: transposed LN stats + rsqrt newton ----
                def ln_stats(xs, g0, ngr, tag):
                    """Per-token rstd and -mean*rstd for token groups
                    [g0, g0+ngr): returned as a [2*ngr, P] f16 tile whose row
                    q holds rstd (q < ngr) / -m*rstd (q >= ngr) for group q."""
                    psT = pss.tile([P, 8], F32, tag="psT", name="psT")
                    sums_b = statp.tile([P, ngr], F32, tag="sumb")
                    scr = statp.tile([P, P], F32, tag="ttrscr")
                    # 4 gram slots in one PSUM bank so group g+1's matmuls
                    # overlap group g's diag extraction
                    gram = psgr.tile([P, 4 * P], F32, tag="gram", name="gram")
                    for g in range(ngr):
                        sl = slice((g0 + g) * P, (g0 + g + 1) * P)
                        gsl = slice((g % 4) * P, (g % 4 + 1) * P)
                        for k in range(KT):
                            nc.tensor.matmul(
                                out=psT[:, g:g + 1], lhsT=xs[k][:, sl],
                                rhs=ones16[:],
                                start=(k == 0), stop=(k == KT - 1))
                        for k in range(KT):
                            nc.tensor.matmul(
                                out=gram[:, gsl], lhsT=xs[k][:, sl],
                                rhs=xs[k][:, sl],
                                start=(k == 0), stop=(k == KT - 1))
                        # sum_x2 = diag(gram) via (gram * I) row-reduce
                        nc.vector.tensor_tensor(
                            out=scr[:], in0=gram[:, gsl], in1=ident16[:],
                            op=OP.mult)
                        nc.vector.tensor_reduce(
                            out=sums_b[:, g:g + 1], in_=scr[:], op=OP.add,
                            axis=mybir.AxisListType.X)
                    m2 = statp.tile([P, ngr], F32, tag="m2")
                    hneg = statp.tile([P, ngr], F32, tag="hneg")
                    y = statp.tile([P, ngr], F32, tag="y")
                    t = statp.tile([P, ngr], F32, tag="t")
                    rsmr = statp.tile([P, 2 * ngr], F16, tag="rsmr")
                    # m2 = sum_x^2/(2 H^2) = m^2/2, all-DVE: stage the PSUM
                    # sums into SBUF first (s2s2d2 forbids two PSUM reads)
                    psTs = statp.tile([P, ngr], F32, tag="psTs")
                    nc.vector.tensor_copy(out=psTs[:], in_=psT[:, 0:ngr])
                    nc.vector.scalar_tensor_tensor(
                        out=m2[:], in0=psTs[:], scalar=0.5 / (H * H),
                        in1=psTs[:], op0=OP.mult, op1=OP.mult)
                    # hneg = m^2/2 - (sum_x2/(2H) + eps/2) = -(var+eps)/2
                    nc.vector.tensor_scalar(
                        out=hneg[:], in0=sums_b[:],
                        scalar1=0.5 / H, scalar2=EPS / 2,
                        op0=OP.mult, op1=OP.add)
                    nc.vector.tensor_sub(hneg[:], m2[:], hneg[:])
                    # rsqrt seed: y = -( (bits(hneg)>>1) - MAGIC2 )
                    nc.vector.tensor_scalar(
                        out=y[:].bitcast(I32), in0=hneg[:].bitcast(I32),
                        scalar1=1, scalar2=None,
                        op0=OP.arith_shift_right)
                    nc.vector.tensor_scalar(
                        out=y[:].bitcast(I32), in0=y[:].bitcast(I32),
                        scalar1=MAGIC2, scalar2=-1, op0=OP.subtract,
                        op1=OP.mult)
                    nc.vector.tensor_mul(t[:], y[:], y[:])
                    nc.vector.tensor_mul(t[:], t[:], hneg[:])
                    nc.vector.scalar_tensor_tensor(
                        out=y[:], in0=t[:], scalar=1.5, in1=y[:],
                        op0=OP.add, op1=OP.mult)
                    nc.vector.tensor_copy(out=rsmr[:, 0:ngr], in_=y[:])
                    # mr = -(sum_x/H)*rstd
                    nc.vector.scalar_tensor_tensor(
                        out=rsmr[:, ngr:2 * ngr], in0=psT[:, 0:ngr],
                        scalar=-1.0 / H, in1=y[:], op0=OP.mult, op1=OP.mult)
                    # transpose each column separately so every row lands
                    # on partition 0 (partition_broadcast requirement)
                    rows = statp.tile([1, 2 * NG * P], F16, tag=f"rows{tag}",
                                      name=f"rows{tag}")
                    for q in range(2 * ngr):
                        ptrq = pst.tile([1, P], F16, tag="pstt", name="ptrq")
                        nc.tensor.transpose(out=ptrq[:], in_=rsmr[:, q:q + 1],
                                            identity=ident16[:])
                        nc.scalar.copy(
                            out=rows[0:1, q * P:(q + 1) * P], in_=ptrq[:])
                    return rows

                def pe_filler(n, rhs_tile):
                    """Keep the PE p-state ramp warm with dead matmuls into
                    a rotating PSUM tile nobody reads."""
                    for _ in range(n):
                        pg = psg.tile([P, 512], F32, tag="pg", name="fill")
                        nc.tensor.matmul(out=pg[:], lhsT=ident16[:],
                                         rhs=rhs_tile[:, 0:512],
                                         start=True, stop=True)

                # ---- phase B: L recurrent layers ----
                for l in range(L):
                    if l == 0:
                        wst[2] = load_wst(2, nc.gpsimd)
                    rows = ln_stats(x, 0, NG, "b")
                    if fa:
                        pe_filler(fa, x[0])

                    # broadcast rstd / -m*rstd across partitions (Pool);
                    # mrb broadcasts overlap the xn multiplies
                    rb = bcp.tile([P, W], F16, tag="rb")
                    mrb = bcp.tile([P, W], F16, tag="mrb")
                    for g in range(NG):
                        nc.gpsimd.partition_broadcast(
                            rb[:, g * P:(g + 1) * P],
                            rows[0:1, g * P:(g + 1) * P])
                    # xn = x*rstd - m*rstd
                    xn = [xnp.tile([P, W], F16, tag=f"xn{k}", name=f"xn{k}")
                          for k in range(KT)]
                    for k in range(KT):
                        nc.vector.tensor_mul(xn[k][:], x[k][:], rb[:])
                    for g in range(NG):
                        nc.gpsimd.partition_broadcast(
                            mrb[:, g * P:(g + 1) * P],
                            rows[0:1, (NG + g) * P:(NG + g + 1) * P])
                    for k in range(KT):
                        nc.vector.tensor_add(xn[k][:], xn[k][:], mrb[:])
                    if fb:
                        pe_filler(fb, x[0])

                    # --- gates GEMM + nonlinearities + scan ---
                    x2 = [xpool.tile([P, W], F16, tag=f"x{k}", name=f"xt{k}")
                          for k in range(KT)]
                    for k in range(KT):
                        sf = work.tile([P, W], F16, tag="sf")
                        si = work.tile([P, W], F16, tag="si")
                        sg = work.tile([P, W], F16, tag="sg")
                        uu = work.tile([P, W], F16, tag="uu")
                        fp = scanp.tile([P, W], F16, tag=f"fp{k}", name=f"fp{k}")
                        mvv = scanp.tile([P, W], F16, tag=f"mv{k}", name=f"mv{k}")

                        def gate_mm(gate, o, n):
                            pg = psg.tile([P, 512], F32, tag="pg")
                            for kk in range(KT):
                                c0 = kk * 3 * H + gate * H + k * P
                                nc.tensor.matmul(
                                    out=pg[:, :n],
                                    lhsT=(wst[l][:, c0:c0 + P]),
                                    rhs=(xn[kk][:, o:o + n]),
                                    start=(kk == 0), stop=(kk == KT - 1))
                            return pg

                        bf = bsg[:, l * 16 + k:l * 16 + k + 1]
                        bi = bsg[:, l * 16 + 4 + k:l * 16 + 4 + k + 1]
                        bt = bsg[:, l * 16 + 8 + k:l * 16 + 8 + k + 1]
                        bth = bsg[:, l * 16 + 12 + k:l * 16 + 12 + k + 1]
                        # chunk1 (512 cols) per gate; the three 128-col
                        # chunk2 regions share one PSUM bank
                        o2 = NCH[1][0]
                        pg2 = psg.tile([P, 512], F32, tag="pg")

                        def gate_mm2(gate):
                            for kk in range(KT):
                                c0 = kk * 3 * H + gate * H + k * P
                                nc.tensor.matmul(
                                    out=pg2[:, gate * P:(gate + 1) * P],
                                    lhsT=(wst[l][:, c0:c0 + P]),
                                    rhs=(xn[kk][:, o2:]),
                                    start=(kk == 0), stop=(kk == KT - 1))

                        pg_f = gate_mm(0, 0, 512)
                        gate_mm2(0)
                        nc.scalar.activation(
                            out=sf[:, 0:512], in_=pg_f[:],
                            func=AF.Sigmoid, bias=bf)
                        nc.scalar.activation(
                            out=sf[:, o2:], in_=pg2[:, 0:P],
                            func=AF.Sigmoid, bias=bf)
                        pg_i = gate_mm(1, 0, 512)
                        gate_mm2(1)
                        nc.scalar.activation(
                            out=si[:, 0:512], in_=pg_i[:],
                            func=AF.Sigmoid, bias=bi)
                        nc.scalar.activation(
                            out=si[:, o2:], in_=pg2[:, P:2 * P],
                            func=AF.Sigmoid, bias=bi)
                        pg_t = gate_mm(2, 0, 512)
                        gate_mm2(2)
                        nc.scalar.activation(
                            out=sg[:, 0:512], in_=pg_t[:],
                            func=AF.Sigmoid, bias=bt)
                        nc.scalar.activation(
                            out=sg[:, o2:], in_=pg2[:, 2 * P:3 * P],
                            func=AF.Sigmoid, bias=bt)
                        nc.scalar.activation(
                            out=uu[:, 0:512], in_=pg_t[:],
                            func=AF.Identity, bias=bth)
                        nc.scalar.activation(
                            out=uu[:, o2:], in_=pg2[:, 2 * P:3 * P],
                            func=AF.Identity, bias=bth)
                        # full-window gate math (one pass per k)
                        ssum = work.tile([P, W], F16, tag="ssum")
                        nc.vector.tensor_add(ssum[:], sf[:], si[:])
                        rinv = work.tile([P, W], F16, tag="rinv")
                        with nc.allow_low_precision("f' in f16 is plenty"):
                            nc.vector.reciprocal(out=rinv[:], in_=ssum[:])
                        nc.vector.tensor_mul(fp[:], sf[:], rinv[:])
                        # g = max(u+0.5, sigmoid(u)); +0.5 folded into uu's
                        # bias so this is a 2x-mode f16 max
                        nc.vector.tensor_max(uu[:], uu[:], sg[:])
                        nc.vector.scalar_tensor_tensor(
                            out=mvv[:], in0=fp[:], scalar=1.0, in1=uu[:],
                            op0=OP.subtract, op1=OP.mult)
                        # boundary reset at own-region start (exact for j==0)
                        t1 = work.tile([P, 1], F32, tag="t1")
                        nc.vector.tensor_mul(
                            t1[:], fp[:, HALO:HALO + 1], rst[:, 1:2])
                        nc.vector.tensor_sub(
                            mvv[:, HALO:HALO + 1], mvv[:, HALO:HALO + 1], t1[:])
                        nc.vector.tensor_mul(
                            fp[:, HALO:HALO + 1], fp[:, HALO:HALO + 1],
                            rst[:, 0:1])
                        # h_t = f'_t h_{t-1} + v_t ;  x2 = h + x
                        nc.vector.tensor_tensor_scan(
                            out=x2[k][:], data0=fp[:], data1=mvv[:],
                            initial=0.5, op0=OP.mult, op1=OP.subtract)
                        nc.vector.tensor_add(x2[k][:], x2[k][:], x[k][:])
                    x = x2

                # ---- final LayerNorm (own tokens = groups 1..4) ----
                rows2 = ln_stats(x, 1, NG - 1, "f")
                if fc:
                    pe_filler(fc, x[0])
                rb2 = bcp.tile([P, CHUNK], F16, tag="rb2")
                mb2 = bcp.tile([P, CHUNK], F16, tag="mb2")
                for g in range(NG - 1):
                    nc.gpsimd.partition_broadcast(
                        rb2[:, g * P:(g + 1) * P],
                        rows2[0:1, g * P:(g + 1) * P])
                    nc.gpsimd.partition_broadcast(
                        mb2[:, g * P:(g + 1) * P],
                        rows2[0:1, (NG - 1 + g) * P:(NG + g) * P])
                for k in range(KT):
                    nc.vector.tensor_mul(xf_bf[k][:], x[k][:, HALO:], rb2[:])
                    nc.vector.tensor_add(xf_bf[k][:], xf_bf[k][:], mb2[:])

            # ---- phase C: logits GEMM (own 512 tokens x full vocab) ----
            # fp8e4m3 DoubleRow, 3 residual-corrected passes:
            #   po = W1@X1 + W1@X2 + W3@X1  with W1 = q8(64*w),
            #   W3 = q8(64*w - W1), X1 = q8(xf), X2 = q8(xf - X1);
            #   logits = po/64 + fc_b   (error ~1.2e-3, see prep)
            x1p = [persist.tile([P, 2, CHUNK], F8, tag=f"x1p{i}", name=f"x1p{i}")
                   for i in range(2)]
            x2p = [persist.tile([P, 2, CHUNK], F8, tag=f"x2p{i}", name=f"x2p{i}")
                   for i in range(2)]
            for k in range(KT):
                i, j = divmod(k, 2)
                nc.vector.tensor_copy(out=x1p[i][:, j, :], in_=xf_bf[k][:])
            for k in range(KT):
                i, j = divmod(k, 2)
                nc.vector.tensor_sub(x2p[i][:, j, :], xf_bf[k][:],
                                     x1p[i][:, j, :])
            VG = 10   # vocab tiles per fcw load (25 groups of 10)
            DR = mybir.MatmulPerfMode.DoubleRow
            with tc.tile_pool(name="osb", bufs=8) as osbp, \
                 tc.tile_pool(name="pso", bufs=8, space="PSUM") as pso:
                for vg in range(25):
                    if prefetch:
                        fcw = fcw_tiles[vg]
                    else:
                        fcw = fcwp.tile([P, 10, 2, 2, 2, P], F8, tag="fcw")
                        nc.gpsimd.dma_start(out=fcw[:], in_=fcwt_t[vg])
                    for j in range(VG):
                        vt = vg * VG + j
                        po = pso.tile([P, CHUNK], F32, tag="po")
                        passes = [(0, x1p), (1, x1p), (0, x2p)]
                        nmm = 0
                        for (t, xs) in passes:
                            for i in range(2):
                                nc.tensor.matmul(
                                    out=po[:], lhsT=fcw[:, j, t, i, :, :],
                                    rhs=xs[i][:],
                                    start=(nmm == 0), stop=(nmm == 5),
                                    perf_mode=DR)
                                nmm += 1
                        jj = vt % 4
                        if jj == 0:
                            osb = osbp.tile([P, 2, 2, CHUNK], F16, tag="osb")
                        if jj % 2 == 0:
                            nc.scalar.activation(
                                out=osb[:, jj // 2, jj % 2, :], in_=po[:],
                                func=AF.Identity, scale=1.0 / 64,
                                bias=fcb2[:, vt:vt + 1])
                        else:
                            nc.vector.tensor_scalar(
                                out=osb[:, jj // 2, jj % 2, :], in0=po[:],
                                scalar1=1.0 / 64,
                                scalar2=fcb2[:, vt:vt + 1],
                                op0=OP.mult, op1=OP.add)
                        if jj == 3 or vt == 249:
                            b0 = (vt - jj) // 2
                            nb = (jj + 1) // 2
                            (nc.scalar if (vt // 4) % 2 == 0
                             else nc.gpsimd).dma_start(
                                out=out_t[:, b0:b0 + nb],
                                in_=osb[:, 0:nb, :, :])

    nc.compile()
    return nc


_CACHED = None


def _get_program():
    global _CACHED
    if _CACHED is None:
        _CACHED = build_program()
    return _CACHED


def prep_inputs(ids, emb, Ws, bs, ln_w, ln_b, fln_w, fc_w, fc_b):
    """Host-side layout prep -> per-core input maps."""
    ids = np.asarray(ids)
    emb = np.asarray(emb, dtype=np.float32)
    Ws = np.asarray(Ws, dtype=np.float32)
    bs = np.asarray(bs, dtype=np.float32)
    ln_w = np.asarray(ln_w, dtype=np.float32)
    ln_b = np.asarray(ln_b, dtype=np.float32)
    fln_w = np.asarray(fln_w, dtype=np.float32)
    fc_w = np.asarray(fc_w, dtype=np.float32)
    fc_b = np.asarray(fc_b, dtype=np.float32)

    emb16 = np.ascontiguousarray(emb).astype(np.float16)

    # fold ln_w into the gate weights, ln_b into the gate biases
    # Ws'[l] = Ws[l] * ln_w[l][None,:]; bias'[l] = bs[l] + Ws[l] @ ln_b[l]
    wsT = np.ascontiguousarray(
        np.stack([(Ws[l] * ln_w[l][None, :]).T.reshape(KT, P, 3 * H)
                  for l in range(L)])).astype(np.float16)
    bias = np.stack([bs[l] + Ws[l] @ ln_b[l] for l in range(L)])  # [L, 3H]

    # per-partition gate biases, grouped [l][kind][k]; kind 3 = bt + 0.5
    bsg = np.empty((P, L * 16), np.float32)
    for l in range(L):
        for gate in range(3):
            for k in range(KT):
                bsg[:, l * 16 + gate * 4 + k] = \
                    bias[l, gate * H + k * P:gate * H + (k + 1) * P]
        for k in range(KT):
            bsg[:, l * 16 + 12 + k] = \
                bias[l, 2 * H + k * P:2 * H + (k + 1) * P] + 0.5

    # fold fln_w into fc_w; quantize to fp8 e4m3 hi+residual at scale 64,
    # tiled [25, P, 10, 2(hi/res), 2(i), 2(j), P]
    import ml_dtypes
    E4 = ml_dtypes.float8_e4m3
    fcw = fc_w * fln_w[None, :]
    w1 = (64.0 * fcw).astype(E4)
    w3 = (64.0 * fcw - w1.astype(np.float32)).astype(E4)

    def _tile8(w8):
        # [H, V] -> [i 2, j 2, c P, vg 25, vt 10, m P] -> [vg, c, vt, i, j, m]
        return w8.T.reshape(2, 2, P, 25, 10, P).transpose(3, 2, 4, 0, 1, 5)

    fcwt = np.ascontiguousarray(
        np.stack([_tile8(w1), _tile8(w3)], axis=3))
    fcb2 = np.ascontiguousarray(fc_b.reshape(V // P, P).T)

    shared = {"emb": emb16, "wsT": wsT, "bsg": bsg,
              "fcwt": fcwt, "fcb": fcb2}

    in_maps = []
    for c in range(N_CORES):
        b, j = divmod(c, 4)
        own0 = j * CHUNK
        win = np.zeros(W, np.int32)
        if j == 0:
            win[HALO:] = ids[b, :CHUNK]
        else:
            win[:] = ids[b, own0 - HALO:own0 + CHUNK]
        idxt = np.ascontiguousarray(win.reshape(NG, P).T)
        rstc = np.empty((P, 2), np.float32)
        rstc[:, 0] = 0.0 if j == 0 else 1.0   # multiplies f at window pos HALO
        rstc[:, 1] = 0.5 if j == 0 else 0.0   # adds f*this to v at pos HALO
        in_maps.append({**shared, "idx": idxt, "rst": rstc})
    return in_maps


def kernel(ids, emb, Ws, bs, ln_w, ln_b, fln_w, fc_w, fc_b):
    nc = _get_program()
    in_maps = prep_inputs(ids, emb, Ws, bs, ln_w, ln_b, fln_w, fc_w, fc_b)
    res = run_bass_kernel_spmd(nc, in_maps, list(range(N_CORES)))
    out = np.empty((B, S, V), np.float32)
    for c in range(N_CORES):
        b, j = divmod(c, 4)
        arr = res.results[c]["out"]  # [P, 125, 2, CHUNK]
        out[b, j * CHUNK:(j + 1) * CHUNK, :] = \
            arr.transpose(3, 1, 2, 0).reshape(CHUNK, V).astype(np.float32)
    return out
